# revision 12
# baseline (speedup 1.0000x reference)
"""Trainium2 Bass kernel for nn_LoLGNN (2-layer hetero GraphSAGE + pooling).

Graph-data parallel over 8 cores; core c owns players [25000c, 25000(c+1))
and graphs [2500c, 2500(c+1)).

Device pipeline (bf16 tables, fp32 psum):
  ENC:  p0 = Xt.T @ Wenc       -- Xt host-packed [cont|emb|1] per core, dense
  HIST: one-hot scatter of a host-permuted dense per-edge feature table
        (48->32-wide combined x/emb features), persisted as sT slabs
  AllGather p0 (bf16), overlapped with HIST
  L1/L2: per stream (teammate/enemy): edges bucketed by (12-window group,
        32K src range, window); batched int16 dma_gather from the allgathered
        bf16 table; fused DVE one-hot (is_equal x mult); PE matmul scatter
        into per-window psum; combine = 5 bf16 matmuls (+DMA-transpose loads
        for the Wr term) -> relu
  Pool: per-window matmul with 0.1-indicators -> pooledT -> Wc
"""
import os
import sys

sys.path.insert(0, "/opt/trn_rl_repo")

import numpy as np
import ml_dtypes

import concourse.bacc as bacc
import concourse.bass as bass
import concourse.tile as tile
import concourse.tile as tile_mod
from concourse import mybir
from concourse.bass_utils import run_bass_kernel_spmd
from bass_rust import ScopedClock, VectorClock

# ---------------------------------------------------------------- constants
N_PLAYER = 200000
N_HIST = 500000
N_GRAPH = 20000
H = 128
NC = 8
PC = N_PLAYER // NC          # players per core
GC = N_GRAPH // NC           # graphs per core
P = 128
NWIN = (PC + P - 1) // P     # 196 dst windows
PCP = NWIN * P               # 25088 padded rows
W_GRP = 12                   # windows per psum group
NG = (NWIN + W_GRP - 1) // W_GRP   # 17 groups
RNG = 32768                  # src range size (int16 gather indices)
NR = (N_PLAYER + RNG - 1) // RNG   # 7 ranges
GCP = 2512                   # padded pooled columns
SUBCH = 8                    # max chunks per dma_gather call (1024-idx ucode cap)
BH = 32                      # hist chunks per load

F32 = mybir.dt.float32
BF16 = mybir.dt.bfloat16
I16 = mybir.dt.int16
BF = ml_dtypes.bfloat16

LAST_EXEC_NS = [None]

# ------------------------------------------------- tail-drain walrus patch
_N_PROCS = 27


def _patched_drain_and_barrier(self, tick_clock, wait_clock):
    gc = tick_clock.global_clock
    nonzero = [p for p in range(_N_PROCS) if gc[p] > 0]
    if not nonzero:
        d = self.nc.sync.drain()
        wait_clock.add_sem_waits(d.ins, ScopedClock({None: gc.copy()}))
    for p in nonzero:
        vec = [0] * _N_PROCS
        vec[p] = gc[p]
        d = self.nc.sync.drain()
        wait_clock.add_sem_waits(d.ins, ScopedClock({None: VectorClock(vec)}))
    self.nc.all_engine_barrier()
    assert self.sems is not None
    popped = self.nc._tile_sem_poison_stack.pop()
    assert popped is self._sem_poison
    self.nc.clear_and_free_semaphores(list(self.sems.allocated().values()))
    self.nc.all_engine_barrier()


tile_mod.TileContext._drain_and_barrier = _patched_drain_and_barrier


# ------------------------------------------------------------- host helpers
def _wrap16(flat_i16):
    """[N] int16 -> [128, N/16] dma_gather layout (16-wrap, 8 Q7 replicas)."""
    a = flat_i16.reshape(-1, 16).T
    return np.tile(a, (8, 1)).copy()


def _prep(inputs):
    f32 = np.float32
    x_player = np.asarray(inputs["x_player"], f32)
    x_history = np.asarray(inputs["x_history"], f32)
    e_tm = np.asarray(inputs["edge_teammate"], np.int64)
    e_en = np.asarray(inputs["edge_enemy"], np.int64)
    e_h = np.asarray(inputs["edge_hist"], np.int64)
    emb_player = np.asarray(inputs["emb_player"], f32)
    emb_h0 = np.asarray(inputs["emb_h0"], f32)
    emb_h3 = np.asarray(inputs["emb_h3"], f32)
    Wp = np.asarray(inputs["Wp"], f32)
    bp = np.asarray(inputs["bp"], f32)
    Wh = np.asarray(inputs["Wh"], f32)
    bh = np.asarray(inputs["bh"], f32)
    sage_Wl = np.asarray(inputs["sage_Wl"], f32)
    sage_b = np.asarray(inputs["sage_b"], f32)
    sage_Wr = np.asarray(inputs["sage_Wr"], f32)
    Wc = np.asarray(inputs["Wc"], f32)
    bc = np.asarray(inputs["bc"], f32)

    # ---- global tables
    ids_p = np.clip(x_player[:, 1:6].astype(np.int64), 0, 199)
    emb80 = np.concatenate(
        [emb_player[k][ids_p[:, k]] for k in range(5)], axis=1)   # [N, 80]

    idh0 = np.clip(x_history[:, 0].astype(np.int64), 0, 1999)
    idh3 = np.clip(x_history[:, 3].astype(np.int64), 0, 9)
    F_h = np.zeros((N_HIST, 32), f32)
    F_h[:, 0:6] = x_history[:, [1, 2, 4, 5, 6, 7]]
    F_h[:, 6:22] = emb_h0[idh0]
    F_h[:, 22:26] = emb_h3[idh3]
    F_h[:, 26] = 1.0          # deg-indicator -> folds Wl2 @ bh per dst

    # ---- weights
    Wenc = np.zeros((97, H), f32)
    for k, c in enumerate([0, 6, 7, 8, 9]):
        Wenc[c] = Wp[:, k]
    Wenc[16:96] = Wp[:, 5:85].T
    Wenc[96] = bp

    Wts = {}
    for l in range(2):
        Wl2 = sage_Wl[l, 2]
        ChT = np.zeros((32, H), f32)
        ChT[0:26] = (Wl2 @ Wh).T
        ChT[26] = Wl2 @ bh
        Wts[l] = dict(
            WlT_tm=sage_Wl[l, 0].T.copy(), WlT_en=sage_Wl[l, 1].T.copy(),
            ChT=ChT, WrT=sage_Wr[l].sum(0).T.copy(),
            bias=sage_b[l].sum(0).reshape(1, H).copy())

    # ---- per-core edge streams (tm=0, en=1)
    cores_ed = []
    for c in range(NC):
        base = c * PC
        per_s = []
        for E in (e_tm, e_en):
            m = (E[1] >= base) & (E[1] < base + PC)
            src = E[0][m]
            dstl = (E[1][m] - base).astype(np.int64)
            deg = np.bincount(dstl, minlength=PC).astype(f32)
            inv = 1.0 / np.maximum(deg, 1.0)
            w = dstl >> 7
            r = src >> 15
            g = w // W_GRP
            key = (g * NR + r) * NWIN + w
            order = np.argsort(key, kind="stable")
            per_s.append((src[order], dstl[order], inv, key[order]))
        cores_ed.append(per_s)

    # bucket counts per core: dict (s, g, r, w) -> n
    counts = [dict() for _ in range(NC)]
    for c in range(NC):
        for s in range(2):
            src, dstl, inv, key = cores_ed[c][s]
            uk, idx0, cnt = np.unique(key, return_index=True,
                                      return_counts=True)
            for k, i0, n in zip(uk, idx0, cnt):
                w = int(k % NWIN)
                gr = int(k // NWIN)
                g, r = gr // NR, gr % NR
                counts[c][(s, g, r, w)] = (int(i0), int(n))

    # union chunk counts
    def wins_of(g):
        return list(range(g * W_GRP, min((g + 1) * W_GRP, NWIN)))

    nch = {}
    for g in range(NG):
        for w in wins_of(g):
            for s in range(2):
                for r in range(NR):
                    mx = 0
                    for c in range(NC):
                        e = counts[c].get((s, g, r, w))
                        if e:
                            mx = max(mx, -(-e[1] // P))
                    if mx:
                        nch[(s, g, r, w)] = mx
                # guarantee >= 1 chunk per (g, w, s) somewhere
                if not any((s, g, r, w) in nch for r in range(NR)):
                    nch[(s, g, 0, w)] = 1

    # registry: per (g, r) ordered chunk list [(w, s, start, stop)], cids
    structure = []          # [g][r] -> list of (wg, s, start, stop)
    cid_base = {}
    cid0_g = []
    TOT = 0
    for g in range(NG):
        cid0_g.append(TOT)
        per_r = []
        for r in range(NR):
            lst = []
            for w in wins_of(g):
                for s in range(2):
                    k = nch.get((s, g, r, w), 0)
                    if k:
                        cid_base[(s, g, r, w)] = TOT + len(lst)
                        lst.extend((w - g * W_GRP, s) for _ in range(k))
            per_r.append(lst)
            TOT += len(lst)
        structure.append(per_r)
    chg = [sum(len(l) for l in structure[g]) for g in range(NG)]
    MAXCHG = max(chg)

    # per-(g,r) sub-call sizes + per-(g,w,s) chain chunk lists
    dstruct = []        # [g][r] = number of chunks (for gather sub-calls)
    chains = []         # [g] = list of (wg, s, [(ci, r, si, col), ...])
    for g in range(NG):
        per_r = [len(structure[g][r]) for r in range(NR)]
        dstruct.append(per_r)
        chl = []
        for w in wins_of(g):
            for s in range(2):
                lst = []
                for r in range(NR):
                    k = nch.get((s, g, r, w), 0)
                    if k:
                        b = cid_base[(s, g, r, w)]
                        # position of chunk b within its (g,r) bucket
                        p0 = b - (cid0_g[g]
                                  + sum(len(structure[g][rr])
                                        for rr in range(r)))
                        for kk in range(k):
                            p = p0 + kk
                            lst.append((b + kk, r, p // SUBCH, p % SUBCH))
                assert lst, (g, w, s)
                chl.append((w - g * W_GRP, s, lst))
        chains.append(chl)

    # per-core idx/dv arrays
    idx_maps, dv_maps = [], []
    for c in range(NC):
        idx_flat = np.zeros(TOT * P, np.int16)
        dst_flat = np.full(TOT * P, -1.0, f32)
        scl_flat = np.zeros(TOT * P, f32)
        for s in range(2):
            src, dstl, inv, key = cores_ed[c][s]
            for (ss, g, r, w), (i0, n) in (
                    (k, v) for k, v in counts[c].items() if k[0] == s):
                b = cid_base[(ss, g, r, w)]
                pos = b * P
                idx_flat[pos:pos + n] = (src[i0:i0 + n] - r * RNG).astype(
                    np.int16)
                dst_flat[pos:pos + n] = (dstl[i0:i0 + n] - w * P).astype(f32)
                scl_flat[pos:pos + n] = inv[dstl[i0:i0 + n]]
        dv = np.empty((P, 2 * TOT), f32)
        dv[:, 0::2] = dst_flat.reshape(TOT, P).T
        dv[:, 1::2] = scl_flat.reshape(TOT, P).T
        idx_maps.append(_wrap16(idx_flat))
        dv_maps.append(dv)

    # ---- hist stream: window-aligned chunks (one window open at a time)
    h_cores = []
    for c in range(NC):
        base = c * PC
        m = (e_h[1] >= base) & (e_h[1] < base + PC)
        src = e_h[0][m]
        dstl = (e_h[1][m] - base).astype(np.int64)
        deg = np.bincount(dstl, minlength=PC).astype(f32)
        inv = 1.0 / np.maximum(deg, 1.0)
        order = np.argsort(dstl, kind="stable")
        src, dstl = src[order], dstl[order]
        wj = dstl >> 7
        i0 = np.searchsorted(wj, np.arange(NWIN))
        i1 = np.searchsorted(wj, np.arange(NWIN), side="right")
        h_cores.append((src, dstl, inv, i0, i1))
    nchw = [max(1, max(-(-int(h[4][w] - h[3][w]) // P) for h in h_cores))
            for w in range(NWIN)]
    hb = np.concatenate([[0], np.cumsum(nchw)])
    NHC = int(hb[-1])
    hjob_list = []
    for w in range(NWIN):
        for k in range(nchw[w]):
            hjob_list.append((int(hb[w] + k), w, k == 0, k == nchw[w] - 1))
    NJH = NHC

    vh_maps, dvh_maps = [], []
    for c in range(NC):
        src, dstl, inv, i0, i1 = h_cores[c]
        Vh = np.zeros((NHC * P, 32), f32)
        dvh = np.empty((P, 2 * NJH), f32)
        dvh[:, 0::2] = -1.0
        dvh[:, 1::2] = 0.0
        for w in range(NWIN):
            n = int(i1[w] - i0[w])
            if n == 0:
                continue
            pos = int(hb[w]) * P
            Vh[pos:pos + n] = F_h[src[i0[w]:i0[w] + n]]
            loc = (dstl[i0[w]:i0[w] + n] - w * P).astype(f32)
            scl = inv[dstl[i0[w]:i0[w] + n]]
            for k in range(-(-n // P)):
                a, b = k * P, min((k + 1) * P, n)
                dvh[0:b - a, 2 * (int(hb[w]) + k)] = loc[a:b]
                dvh[0:b - a, 2 * (int(hb[w]) + k) + 1] = scl[a:b]
        vh_maps.append(Vh.astype(BF))
        dvh_maps.append(dvh)

    # ---- pooling indicators [128, NWIN*16]
    ind = np.zeros((P, NWIN * 16), f32)
    gbase = []
    for w in range(NWIN):
        g0 = (w * P) // 10
        gbase.append(g0)
        for d in range(min(P, PC - w * P)):
            gi = (w * P + d) // 10 - g0
            if gi < 16:
                ind[d, w * 16 + gi] = 0.1

    # ---- per-core input maps
    in_maps = []
    iota = np.tile(np.arange(P, dtype=f32), (P, 1))
    for c in range(NC):
        base = c * PC
        Xt = np.zeros((97, PCP), f32)
        Xt[0:10, :PC] = x_player[base:base + PC].T
        Xt[16:96, :PC] = emb80[base:base + PC].T
        Xt[96, :PC] = 1.0
        m = dict(
            Xt=Xt.astype(BF),
            Vh=vh_maps[c],
            idxg=idx_maps[c],
            dv=dv_maps[c],
            dvh=dvh_maps[c],
            iota=iota.astype(BF),
            Wenc=Wenc.astype(BF),
            onesrow=np.ones((1, P), f32).astype(BF),
            poolind=ind.astype(BF),
            WcT=Wc.T.copy().astype(BF),
        )
        for l in range(2):
            m[f"WlT_tm_{l}"] = Wts[l]["WlT_tm"].astype(BF)
            m[f"WlT_en_{l}"] = Wts[l]["WlT_en"].astype(BF)
            m[f"ChT_{l}"] = Wts[l]["ChT"].astype(BF)
            m[f"WrT_{l}"] = Wts[l]["WrT"].astype(BF)
            m[f"bias_{l}"] = Wts[l]["bias"].astype(BF)
        in_maps.append(m)

    rsz = [min(RNG, N_PLAYER - r * RNG) for r in range(NR)]
    cfg = dict(dstruct=dstruct, chains=chains, chg=chg, cid0=cid0_g,
               TOT=TOT, MAXCHG=MAXCHG, NHC=NHC, NJH=NJH, hjobs=hjob_list,
               rsz=rsz, gbase=gbase, bc=float(bc[0]))
    return in_maps, cfg


# ------------------------------------------------------------ device build
def _build(cfg):
    dstruct = cfg["dstruct"]
    chains = cfg["chains"]
    chg = cfg["chg"]
    cid0 = cfg["cid0"]
    TOT = cfg["TOT"]
    MAXCHG = cfg["MAXCHG"]
    NHC = cfg["NHC"]
    NJH = cfg["NJH"]
    hjobs = cfg["hjobs"]
    rsz = cfg["rsz"]
    gbase = cfg["gbase"]

    nc = bacc.Bacc("TRN2", target_bir_lowering=False, debug=False,
                   num_devices=NC, dynamic_dma_scratch_size=65536)

    dram_in = {}
    for name, shp, dt in [
            ("Xt", [97, PCP], BF16), ("Vh", [NHC * P, 32], BF16),
            ("idxg", [P, TOT * 8], I16), ("dv", [P, 2 * TOT], F32),
            ("dvh", [P, 2 * NJH], F32), ("iota", [P, P], BF16),
            ("Wenc", [97, H], BF16), ("onesrow", [1, P], BF16),
            ("poolind", [P, NWIN * 16], BF16), ("WcT", [H, 1], BF16)]:
        dram_in[name] = nc.dram_tensor(name, shp, dt, kind="ExternalInput")
    for l in range(2):
        for name, shp in [(f"WlT_tm_{l}", [H, H]), (f"WlT_en_{l}", [H, H]),
                          (f"ChT_{l}", [32, H]), (f"WrT_{l}", [H, H]),
                          (f"bias_{l}", [1, H])]:
            dram_in[name] = nc.dram_tensor(name, shp, BF16,
                                           kind="ExternalInput")
    y_out = nc.dram_tensor("y", [1, GC], F32, kind="ExternalOutput")
    debug = bool(os.environ.get("GNN_DEBUG"))
    dbg = {}
    if debug:
        for name, shp, dt in [("dbg_p0", [PCP, H], BF16),
                              ("dbg_sTh", [32, PCP], BF16),
                              ("dbg_p1", [PCP, H], BF16),
                              ("dbg_pool", [P, GCP], F32)]:
            dbg[name] = nc.dram_tensor(name, shp, dt, kind="ExternalOutput")

    def wins_of(g):
        return list(range(g * W_GRP, min((g + 1) * W_GRP, NWIN)))

    with tile.TileContext(nc) as tc, \
         tc.tile_pool(name="const", bufs=1) as constp, \
         tc.tile_pool(name="xt", bufs=2) as xtp, \
         tc.tile_pool(name="meta", bufs=2) as metap, \
         tc.tile_pool(name="v", bufs=1) as vp, \
         tc.tile_pool(name="vh", bufs=2) as vhp, \
         tc.tile_pool(name="oh", bufs=4) as ohp, \
         tc.tile_pool(name="st", bufs=2) as stp, \
         tc.tile_pool(name="tr", bufs=2) as trp, \
         tc.tile_pool(name="ot", bufs=3) as otp, \
         tc.tile_pool(name="acc", bufs=1) as accp, \
         tc.tile_pool(name="agg", bufs=6, space="PSUM") as aggp, \
         tc.tile_pool(name="scr", bufs=2, space="PSUM") as scrp, \
         tc.tile_pool(name="dram", bufs=1, space="DRAM") as dramp:

        # ---- constants
        C = {}
        for name in ["iota", "Wenc", "onesrow", "poolind", "WcT"] + \
                [f"{w}_{l}" for l in range(2)
                 for w in ["WlT_tm", "WlT_en", "ChT", "WrT", "bias"]]:
            t = constp.tile(list(dram_in[name].shape), dram_in[name].dtype,
                            tag=f"c_{name}")
            nc.sync.dma_start(t[:], dram_in[name][:])
            C[name] = t

        pooledT = accp.tile([P, GCP], F32, tag="pooledT")
        nc.vector.memset(pooledT[:], 0.0)

        # ---- DRAM intermediates
        p0_pad = dramp.tile([PCP, H], BF16)
        p1_pad = dramp.tile([PCP, H], BF16)
        p0_full = dramp.tile([N_PLAYER, H], BF16)
        p1_full = dramp.tile([N_PLAYER, H], BF16)
        sTh_d = dramp.tile([32, PCP], BF16)

        # ================= ENC =================
        nslab = (NWIN + 7) // 8
        for sl in range(nslab):
            w0 = sl * 8
            nw = min(8, NWIN - w0)
            xt = xtp.tile([97, 8 * P], BF16, tag="xt")
            nc.sync.dma_start(xt[:, :nw * P],
                              dram_in["Xt"][:, w0 * P:(w0 + nw) * P])
            for wo in range(nw):
                pot = scrp.tile([P, 512], F32, tag="po", name="po")
                po = pot[:, 0:H]
                nc.tensor.matmul(po, lhsT=xt[:, wo * P:(wo + 1) * P],
                                 rhs=C["Wenc"][:], start=True, stop=True)
                ot = otp.tile([P, H], BF16, tag="ot")
                nc.scalar.copy(ot[:], po)
                nc.scalar.dma_start(
                    p0_pad[(w0 + wo) * P:(w0 + wo + 1) * P, :], ot[:])

        if debug:
            nc.sync.dma_start(dbg["dbg_p0"][:, :], p0_pad[:, :])
        # ================= AllGather p0 =================
        nc.gpsimd.collective_compute(
            "AllGather", mybir.AluOpType.bypass,
            replica_groups=[list(range(NC))],
            ins=[p0_pad[0:PC, :]], outs=[p0_full.opt()])

        # ================= HIST =================
        dvh_sb = accp.tile([P, 2 * NJH], F32, tag="dvh_sb")
        nc.sync.dma_start(dvh_sb[:], dram_in["dvh"][:])
        sThbuf = None
        cur_hg = -1
        psum_h = {}
        ji = 0
        nload = (NHC + BH - 1) // BH
        for b in range(nload):
            c0 = b * BH
            nb = min(BH, NHC - c0)
            vh = vhp.tile([P, BH, 32], BF16, tag="vh")
            src_ap = dram_in["Vh"][c0 * P:(c0 + nb) * P, :].rearrange(
                "(c e) f -> e c f", e=P)
            nc.sync.dma_start(vh[:, :nb, :], src_ap)
            while ji < NJH and hjobs[ji][0] < c0 + nb:
                j, w, st, sp = hjobs[ji]
                oh = ohp.tile([P, P], BF16, tag="oh")
                nc.vector.tensor_scalar(
                    out=oh[:], in0=C["iota"][:],
                    scalar1=dvh_sb[:, 2 * ji:2 * ji + 1],
                    scalar2=dvh_sb[:, 2 * ji + 1:2 * ji + 2],
                    op0=mybir.AluOpType.is_equal, op1=mybir.AluOpType.mult)
                if st:
                    psum_h[w] = scrp.tile([P, 512], F32, tag="po",
                                          name="po")[0:32, 0:P]
                nc.tensor.matmul(psum_h[w], lhsT=vh[:, j - c0, :],
                                 rhs=oh[:], start=st, stop=sp)
                if sp:
                    hg = w // W_GRP
                    if hg != cur_hg:
                        if sThbuf is not None:
                            g0w = cur_hg * W_GRP
                            nwv = len(wins_of(cur_hg))
                            nc.scalar.dma_start(
                                sTh_d[:, g0w * P:(g0w + nwv) * P],
                                sThbuf[:, :nwv * P])
                        sThbuf = stp.tile([32, W_GRP * P], BF16, tag="sThbuf")
                        cur_hg = hg
                    nc.scalar.copy(
                        sThbuf[:, (w % W_GRP) * P:(w % W_GRP + 1) * P],
                        psum_h[w])
                    del psum_h[w]
                ji += 1
        g0w = cur_hg * W_GRP
        nwv = len(wins_of(cur_hg))
        nc.scalar.dma_start(sTh_d[:, g0w * P:(g0w + nwv) * P],
                            sThbuf[:, :nwv * P])

        if debug:
            nc.sync.dma_start(dbg["dbg_sTh"][:, :], sTh_d[:, :])
        # ================= layers =================
        for l in range(2):
            table = p0_full if l == 0 else p1_full
            p_prev = p0_pad if l == 0 else p1_pad
            for g in range(NG):
                wl = wins_of(g)
                nwg = len(wl)
                cg = chg[g]
                idxt = metap.tile([P, MAXCHG * 8], I16, tag="idxt")
                nc.sync.dma_start(
                    idxt[:, :cg * 8],
                    dram_in["idxg"][:, cid0[g] * 8:(cid0[g] + cg) * 8])
                dvt = metap.tile([P, 2 * MAXCHG], F32, tag="dvt")
                nc.sync.dma_start(
                    dvt[:, :2 * cg],
                    dram_in["dv"][:, 2 * cid0[g]:2 * (cid0[g] + cg)])

                # phase 1: all gathers for this group (V tiles stay live)
                vtiles = {}
                cpos = 0
                for r in range(NR):
                    nck = dstruct[g][r]
                    r0 = r * RNG
                    for si in range(-(-nck // SUBCH) if nck else 0):
                        ns = min(SUBCH, nck - si * SUBCH)
                        V = vp.tile([P, SUBCH, P], BF16, tag=f"V{r}_{si}",
                                    name=f"V{r}_{si}")
                        nc.gpsimd.dma_gather(
                            out_ap=V[:, :ns, :],
                            in_ap=table[r0:r0 + rsz[r], :],
                            idxs_ap=idxt[:, cpos * 8:(cpos + ns) * 8],
                            num_idxs=ns * P, num_idxs_reg=ns * P,
                            elem_size=P, single_packet=False)
                        vtiles[(r, si)] = V
                        cpos += ns
                # phase 2: contiguous accumulation chain per (window, stream)
                agg = {}
                for (wg, s, lst) in chains[g]:
                    ps = aggp.tile([P, P], F32, tag="aggs", name="aggs")
                    agg[(wg, s)] = ps
                    nj = len(lst)
                    for i, (ci, r, si, col) in enumerate(lst):
                        cig = ci - cid0[g]
                        oh = ohp.tile([P, P], BF16, tag="oh")
                        nc.vector.tensor_scalar(
                            out=oh[:], in0=C["iota"][:],
                            scalar1=dvt[:, 2 * cig:2 * cig + 1],
                            scalar2=dvt[:, 2 * cig + 1:2 * cig + 2],
                            op0=mybir.AluOpType.is_equal,
                            op1=mybir.AluOpType.mult)
                        nc.tensor.matmul(agg[(wg, s)][:],
                                         lhsT=vtiles[(r, si)][:, col, :],
                                         rhs=oh[:], start=(i == 0),
                                         stop=(i == nj - 1))

                # ---- combine for this group
                ptg = trp.tile([P, W_GRP * P], BF16, tag="ptg")
                nc.sync.dma_start(ptg[:, :nwg * P],
                                  p_prev[wl[0] * P:(wl[0] + nwg) * P, :],
                                  transpose=True)
                shg = trp.tile([32, W_GRP * P], BF16, tag="shg")
                nc.scalar.dma_start(shg[:, :nwg * P],
                                    sTh_d[:, wl[0] * P:(wl[0] + nwg) * P])
                stb = stp.tile([P, 2 * W_GRP * P], BF16, tag="stb")
                for wg, w in enumerate(wl):
                    nc.scalar.copy(stb[:, (2 * wg) * P:(2 * wg + 1) * P],
                                   agg[(wg, 0)][:])
                    nc.scalar.copy(stb[:, (2 * wg + 1) * P:(2 * wg + 2) * P],
                                   agg[(wg, 1)][:])
                    pot = scrp.tile([P, 512], F32, tag="po", name="po")
                    po = pot[:, 0:H]
                    nc.tensor.matmul(
                        po, lhsT=stb[:, (2 * wg) * P:(2 * wg + 1) * P],
                        rhs=C[f"WlT_tm_{l}"][:], start=True, stop=False)
                    nc.tensor.matmul(
                        po, lhsT=stb[:, (2 * wg + 1) * P:(2 * wg + 2) * P],
                        rhs=C[f"WlT_en_{l}"][:], start=False, stop=False)
                    nc.tensor.matmul(
                        po, lhsT=shg[:, wg * P:(wg + 1) * P],
                        rhs=C[f"ChT_{l}"][:], start=False, stop=False)
                    nc.tensor.matmul(
                        po, lhsT=ptg[:, wg * P:(wg + 1) * P],
                        rhs=C[f"WrT_{l}"][:], start=False, stop=False)
                    nc.tensor.matmul(
                        po, lhsT=C["onesrow"][:], rhs=C[f"bias_{l}"][:],
                        start=False, stop=True)
                    ot = otp.tile([P, H], BF16, tag="ot")
                    nc.scalar.activation(ot[:], po,
                                         mybir.ActivationFunctionType.Relu)
                    if l == 0:
                        nc.scalar.dma_start(p1_pad[w * P:(w + 1) * P, :],
                                            ot[:])
                    else:
                        pp = scrp.tile([P, 512], F32, tag="po",
                                       name="po")[:, 0:16]
                        nc.tensor.matmul(
                            pp, lhsT=ot[:],
                            rhs=C["poolind"][:, w * 16:(w + 1) * 16],
                            start=True, stop=True)
                        g0 = gbase[w]
                        nc.vector.tensor_tensor(
                            out=pooledT[:, g0:g0 + 16],
                            in0=pooledT[:, g0:g0 + 16], in1=pp,
                            op=mybir.AluOpType.add)
            if l == 0:
                nc.gpsimd.collective_compute(
                    "AllGather", mybir.AluOpType.bypass,
                    replica_groups=[list(range(NC))],
                    ins=[p1_pad[0:PC, :]], outs=[p1_full.opt()])

        if debug:
            nc.sync.dma_start(dbg["dbg_p1"][:, :], p1_pad[:, :])
            nc.sync.dma_start(dbg["dbg_pool"][:, :], pooledT[:])
        # ================= output =================
        pooledbf = accp.tile([P, GCP], BF16, tag="pooledbf")
        nc.vector.tensor_copy(pooledbf[:], pooledT[:])
        yrow = accp.tile([1, GC], F32, tag="yrow")
        for k0 in range(0, GC, 512):
            kn = min(512, GC - k0)
            ps = scrp.tile([P, 512], F32, tag="po", name="po")[0:1, :]
            nc.tensor.matmul(ps[:, :kn], lhsT=C["WcT"][:],
                             rhs=pooledbf[:, k0:k0 + kn],
                             start=True, stop=True)
            nc.scalar.add(yrow[:, k0:k0 + kn], ps[:, :kn], cfg["bc"])
        nc.sync.dma_start(y_out[:, :], yrow[:])

    nc.compile()
    return nc


def kernel(**inputs):
    in_maps, cfg = _prep(inputs)
    nc = _build(cfg)
    trace = bool(os.environ.get("GNN_TRACE"))
    res = run_bass_kernel_spmd(nc, in_maps, core_ids=list(range(NC)),
                               trace=trace)
    LAST_EXEC_NS[0] = res.exec_time_ns
    out = np.concatenate([np.asarray(res.results[c]["y"]).reshape(GC, 1)
                          for c in range(NC)], axis=0)
    return out.astype(np.float32)
